# revision 16
# baseline (speedup 1.0000x reference)
"""Block-diagonal dense (nn_BlockDiagonalDense) Trainium2 Bass kernel.

Math: x [B=4, T=4096, F=4096] fp32; per token, features are grouped into
512 blocks of 8; each block is multiplied by its own 8x8 matrix
(kernel [16 heads, 32 blocks, 8, 8]) and bias added (bias is zeros in
setup_inputs, but we fold it in anyway).

Strategy:
  - Data-parallel over tokens across 8 cores (16384 tokens -> 2048/core).
  - Weights are expanded host-side into 32 chunks of 128x128 block-diagonal
    matrices (one per 128 consecutive features), replicated to every core.
  - On-chip per 128-token tile: PE transpose of each 128-feature chunk
    (fp32, via identity matmul) -> PSUM -> copy to SBUF (ScalarE) ->
    PE matmul lhsT=x^T chunk (stationary), rhs=W chunk (moving) giving
    token-major output in PSUM -> VectorE drain with fused bias add ->
    contiguous DMA out.
"""

import sys

if "/opt/trn_rl_repo" not in sys.path:
    sys.path.insert(0, "/opt/trn_rl_repo")

import numpy as np

NUM_HEADS = 16
BLOCK_SIZE = 8
FEATURES = 4096
HEAD_DIM = FEATURES // NUM_HEADS  # 256
BLOCK_DIM = HEAD_DIM // BLOCK_SIZE  # 32

N_CORES = 8
TOKENS_TOTAL = 4 * 4096  # 16384
TOK_PER_CORE = TOKENS_TOTAL // N_CORES  # 2048

P = 128  # partitions
N_CHUNKS = FEATURES // P  # 32 chunks of 128 features
CG = 4  # chunks per group (512 output cols per PSUM bank)

_NC_CACHE = {}


def build_nc(
    tok_per_core=TOK_PER_CORE,
    repeats=1,
    dma_pattern="split",
    edge_split=True,
    xt_engine="scalar",
    edge_dual=False,
    cg=CG,
    pst_bufs=3,
    psy_bufs=3,
    xbufs=4,
    ybufs=4,
    xtbufs=4,
    mm_dtype="bf16",
    xin_dtype="f32",
):
    """Build the Bass program for one core processing [tok_per_core, 4096].

    repeats>1 wraps the whole body in a hardware loop doing identical work
    (same inputs, same outputs) -- used only for slope-based device timing.

    dma_pattern: "split" = x on SP ring / y on ACT ring;
                 "alt2"  = both rings alternate directions per tile;
                 "alt3"  = SP + ACT + SWDGE(gpsimd) rotate.
    """
    import contextlib

    import concourse.bass as bass
    import concourse.mybir as mybir
    from concourse import bacc
    from concourse.masks import make_identity
    from concourse.tile import TileContext

    f32 = mybir.dt.float32
    wdt = mybir.dt.bfloat16 if mm_dtype == "bf16" else f32
    xdt = mybir.dt.bfloat16 if xin_dtype == "bf16" else f32
    nc = bacc.Bacc(None, target_bir_lowering=False)

    x = nc.declare_dram_parameter("x", [tok_per_core, FEATURES], f32, isOutput=False)
    # w: [128 (fi within chunk), 32*128 (chunk-major, fo within chunk)]
    w = nc.declare_dram_parameter("w", [P, N_CHUNKS * P], wdt, isOutput=False)
    b = nc.declare_dram_parameter("b", [FEATURES], f32, isOutput=False)
    y = nc.declare_dram_parameter("y", [tok_per_core, FEATURES], f32, isOutput=True)

    n_tiles = tok_per_core // P

    with TileContext(nc) as tc:
        with (
            tc.tile_pool(name="const", bufs=1) as const_pool,
            tc.tile_pool(name="xin", bufs=xbufs) as x_pool,
            tc.tile_pool(name="yout", bufs=ybufs) as y_pool,
            tc.tile_pool(name="xt", bufs=xtbufs) as xt_pool,
            tc.tile_pool(name="pst", bufs=pst_bufs, space="PSUM") as pst_pool,
            tc.tile_pool(name="psy", bufs=psy_bufs, space="PSUM") as psy_pool,
        ):
            # w on the ACT ring: keeps tile-0's x DMA unqueued on the SP ring
            w_sb = const_pool.tile([P, N_CHUNKS * P], wdt)
            nc.scalar.dma_start(out=w_sb, in_=w[:, :])

            # bias replicated across all 128 partitions (partition-stride 0)
            bias_sb = const_pool.tile([P, FEATURES], f32)
            b_ap = b[:]
            bias_bcast = bass.AP(
                tensor=b_ap.tensor, offset=b_ap.offset, ap=[[0, P], [1, FEATURES]]
            )
            nc.gpsimd.dma_start(out=bias_sb, in_=bias_bcast)

            ident = const_pool.tile([P, P], xdt)
            make_identity(nc, ident)

            rep_ctx = (
                tc.For_i(0, repeats, 1) if repeats > 1 else contextlib.nullcontext()
            )
            if xin_dtype == "bf16":
                # fp32->bf16 cast during DMA needs SWDGE (gpsimd); spread the
                # fp32 stores across both HWDGE rings
                in_engines, out_engines = (nc.gpsimd,), (nc.sync, nc.scalar)
            elif dma_pattern == "split":
                in_engines, out_engines = (nc.sync,), (nc.scalar,)
            elif dma_pattern == "alt2":
                in_engines, out_engines = (nc.sync, nc.scalar), (nc.scalar, nc.sync)
            elif dma_pattern == "alt3":
                in_engines = (nc.sync, nc.gpsimd, nc.scalar)
                out_engines = (nc.scalar, nc.sync, nc.gpsimd)
            elif dma_pattern == "dual":
                # both directions split as column-halves across both rings
                in_engines, out_engines = (nc.sync,), (nc.scalar,)
            else:
                raise ValueError(dma_pattern)

            with rep_ctx:
                for ti in range(n_tiles):
                    x_tile = x_pool.tile([P, FEATURES], xdt)
                    rows = slice(ti * P, (ti + 1) * P)
                    if ti == 0 and edge_split:
                        # split the pipeline-head DMA across BOTH rings so
                        # chunk-0 compute starts after the first quarter
                        Q = FEATURES // 4
                        for q in range(4):
                            ((nc.sync, nc.scalar)[q % 2] if edge_dual else in_engines[q % len(in_engines)]).dma_start(
                                out=x_tile[:, q * Q : (q + 1) * Q],
                                in_=x[rows, q * Q : (q + 1) * Q],
                            )
                    elif dma_pattern == "dual":
                        if xin_dtype == "bf16":
                            nc.gpsimd.dma_start(out=x_tile, in_=x[rows, :])
                        else:
                            H = FEATURES // 2
                            nc.sync.dma_start(out=x_tile[:, :H], in_=x[rows, :H])
                            nc.scalar.dma_start(out=x_tile[:, H:], in_=x[rows, H:])
                    else:
                        in_engines[ti % len(in_engines)].dma_start(
                            out=x_tile, in_=x[rows, :]
                        )

                    y_tile = y_pool.tile([P, FEATURES], f32)

                    for g in range(N_CHUNKS // cg):
                        # bf16 ps_t is padded to a full 2KB PSUM bank so pool
                        # bufs never share a bank (PE-write vs ACT-read on the
                        # same bank is a fatal HW collision)
                        if xdt == f32:
                            ps_t = pst_pool.tile([P, cg * P], f32)
                        else:
                            ps_t_full = pst_pool.tile([P, 2 * cg * P], xdt)
                            ps_t = ps_t_full[:, : cg * P]
                        for k in range(cg):
                            c = g * cg + k
                            nc.tensor.transpose(
                                ps_t[:, k * P : (k + 1) * P],
                                x_tile[:, c * P : (c + 1) * P],
                                ident,
                            )
                        xt = xt_pool.tile([P, cg * P], wdt)
                        if xt_engine == "scalar":
                            nc.scalar.copy(xt, ps_t)
                        else:
                            nc.vector.tensor_copy(xt, ps_t)

                        ps_y = psy_pool.tile([P, cg * P], f32)
                        for k in range(cg):
                            c = g * cg + k
                            nc.tensor.matmul(
                                ps_y[:, k * P : (k + 1) * P],
                                xt[:, k * P : (k + 1) * P],
                                w_sb[:, c * P : (c + 1) * P],
                            )
                        # drain + fused bias add (bias varies along free dim)
                        nc.vector.tensor_add(
                            y_tile[:, g * cg * P : (g + 1) * cg * P],
                            ps_y,
                            bias_sb[:, g * cg * P : (g + 1) * cg * P],
                        )

                    # out-DMA off the input ring so both directions overlap
                    if ti == n_tiles - 1 and edge_split:
                        # split the pipeline-tail DMA across BOTH rings so
                        # stores begin as soon as the first chunk groups drain
                        Q = FEATURES // 4
                        for q in range(4):
                            ((nc.scalar, nc.sync)[q % 2] if edge_dual else out_engines[q % len(out_engines)]).dma_start(
                                out=y[rows, q * Q : (q + 1) * Q],
                                in_=y_tile[:, q * Q : (q + 1) * Q],
                            )
                    elif dma_pattern == "dual":
                        H = FEATURES // 2
                        nc.scalar.dma_start(out=y[rows, :H], in_=y_tile[:, :H])
                        nc.sync.dma_start(out=y[rows, H:], in_=y_tile[:, H:])
                    else:
                        out_engines[ti % len(out_engines)].dma_start(
                            out=y[rows, :], in_=y_tile
                        )

    nc.finalize()
    return nc


def build_nc_alt2(**kw):
    return build_nc(dma_pattern="alt2", **kw)


def expand_weights(kern, mm_dtype="bf16"):
    """kernel [16, 32, 8, 8] -> [128, 32*128] chunk-major block-diagonal."""
    kern = np.asarray(kern, dtype=np.float32)
    wd = np.zeros((N_CHUNKS, P, P), dtype=np.float32)
    for c in range(N_CHUNKS):
        h = c // 2
        for j in range(16):
            bd = 16 * (c % 2) + j
            wd[c, 8 * j : 8 * j + 8, 8 * j : 8 * j + 8] = kern[h, bd]
    # [chunk, fi, fo] -> [fi, chunk*128 + fo]
    w = np.ascontiguousarray(wd.transpose(1, 0, 2).reshape(P, N_CHUNKS * P))
    if mm_dtype == "bf16":
        import ml_dtypes

        w = w.astype(ml_dtypes.bfloat16)
    return w


def reference_numpy(x, kern, bias):
    xb = np.asarray(x, np.float32).reshape(-1, NUM_HEADS, BLOCK_DIM, BLOCK_SIZE)
    k = np.asarray(kern, np.float32)
    y = np.einsum("nhbs,hbst->nhbt", xb, k) + np.asarray(bias, np.float32)
    return y.reshape(x.shape)


_LAST_EXEC_NS = None


def kernel(**inputs):
    """Full inputs in, full output out. Shards tokens across 8 cores."""
    global _LAST_EXEC_NS
    import os

    from concourse.bass_utils import run_bass_kernel_spmd

    x = np.ascontiguousarray(np.asarray(inputs["x"], dtype=np.float32))
    kern = np.asarray(inputs["kernel"], dtype=np.float32)
    bias = np.ascontiguousarray(
        np.asarray(inputs["bias"], dtype=np.float32).reshape(FEATURES)
    )

    orig_shape = x.shape
    xf = x.reshape(TOKENS_TOTAL, FEATURES)
    w = expand_weights(kern)

    if "nc" not in _NC_CACHE:
        _NC_CACHE["nc"] = build_nc()
    nc = _NC_CACHE["nc"]

    in_maps = [
        {
            "x": xf[c * TOK_PER_CORE : (c + 1) * TOK_PER_CORE],
            "w": w,
            "b": bias,
        }
        for c in range(N_CORES)
    ]

    trace = bool(os.environ.get("BASS_KERNEL_TRACE"))
    res = run_bass_kernel_spmd(nc, in_maps, list(range(N_CORES)), trace=trace)
    _LAST_EXEC_NS = res.exec_time_ns

    y = np.concatenate([r["y"] for r in res.results], axis=0)
    return y.reshape(orig_shape)



# revision 23
# speedup vs baseline: 1.0083x; 1.0083x over previous
"""Block-diagonal dense (nn_BlockDiagonalDense) Trainium2 Bass kernel.

Math: x [B=4, T=4096, F=4096] fp32; per token, features are grouped into
512 blocks of 8; each block is multiplied by its own 8x8 matrix
(kernel [16 heads, 32 blocks, 8, 8]) and bias added (bias is zeros in
setup_inputs, but we fold it in anyway).

Strategy:
  - Data-parallel over tokens across 8 cores (16384 tokens -> 2048/core).
  - Weights are expanded host-side into 32 chunks of 128x128 block-diagonal
    matrices (one per 128 consecutive features), replicated to every core
    in bf16 (tolerance is 2e-2; bf16 keeps rel err ~2e-3 and runs the PE
    at 1 cycle/row instead of fp32's 4).
  - On-chip per 128-token tile: PE transpose of each 128-feature chunk
    (fp32, via identity matmul) -> PSUM -> cast-copy to bf16 SBUF
    (ScalarE) -> PE matmul lhsT=x^T chunk (stationary bf16), rhs=W chunk
    (moving bf16) giving token-major fp32 output in PSUM -> VectorE drain
    with fused bias add -> contiguous DMA out.
  - The workload is HBM-bound (2MB in + 2MB out per tile vs ~358 GB/s
    per-core HBM); DMA pattern selection targets steady full-rate
    streaming in both directions.
"""

import sys

if "/opt/trn_rl_repo" not in sys.path:
    sys.path.insert(0, "/opt/trn_rl_repo")

import numpy as np

NUM_HEADS = 16
BLOCK_SIZE = 8
FEATURES = 4096
HEAD_DIM = FEATURES // NUM_HEADS  # 256
BLOCK_DIM = HEAD_DIM // BLOCK_SIZE  # 32

N_CORES = 8
TOKENS_TOTAL = 4 * 4096  # 16384
TOK_PER_CORE = TOKENS_TOTAL // N_CORES  # 2048

P = 128  # partitions
N_CHUNKS = FEATURES // P  # 32 chunks of 128 features
CG = 4  # chunks per group (512 output cols per PSUM bank)

_NC_CACHE = {}


def build_nc(
    tok_per_core=TOK_PER_CORE,
    repeats=1,
    dma_pattern="split",
    edge_split=True,
    xt_engine="scalar",
    edge_dual=False,
    cg=CG,
    pst_bufs=3,
    psy_bufs=3,
    xbufs=4,
    ybufs=4,
    xtbufs=4,
    mm_dtype="bf16",
    xin_dtype="f32",
    out_delay=2,
):
    """Build the Bass program for one core processing [tok_per_core, 4096].

    repeats>1 wraps the whole body in a hardware loop doing identical work
    (same inputs, same outputs) -- used only for slope-based device timing.

    dma_pattern: "split" = x on SP ring / y on ACT ring;
                 "alt2"  = both rings alternate directions per tile;
                 "alt3"  = SP + ACT + SWDGE(gpsimd) rotate.
    """
    import contextlib

    import concourse.bass as bass
    import concourse.mybir as mybir
    from concourse import bacc
    from concourse.masks import make_identity
    from concourse.tile import TileContext

    f32 = mybir.dt.float32
    wdt = mybir.dt.bfloat16 if mm_dtype == "bf16" else f32
    xdt = mybir.dt.bfloat16 if xin_dtype == "bf16" else f32
    nc = bacc.Bacc(None, target_bir_lowering=False)

    x = nc.declare_dram_parameter("x", [tok_per_core, FEATURES], f32, isOutput=False)
    # w: [128 (fi within chunk), 32*128 (chunk-major, fo within chunk)]
    w = nc.declare_dram_parameter("w", [P, N_CHUNKS * P], wdt, isOutput=False)
    b = nc.declare_dram_parameter("b", [FEATURES], f32, isOutput=False)
    y = nc.declare_dram_parameter("y", [tok_per_core, FEATURES], f32, isOutput=True)

    n_tiles = tok_per_core // P

    with TileContext(nc) as tc:
        with (
            tc.tile_pool(name="const", bufs=1) as const_pool,
            tc.tile_pool(name="xin", bufs=xbufs) as x_pool,
            tc.tile_pool(name="yout", bufs=ybufs) as y_pool,
            tc.tile_pool(name="xt", bufs=xtbufs) as xt_pool,
            tc.tile_pool(name="pst", bufs=pst_bufs, space="PSUM") as pst_pool,
            tc.tile_pool(name="psy", bufs=psy_bufs, space="PSUM") as psy_pool,
        ):
            # w on the ACT ring: keeps tile-0's x DMA unqueued on the SP ring
            w_sb = const_pool.tile([P, N_CHUNKS * P], wdt)
            nc.scalar.dma_start(out=w_sb, in_=w[:, :])

            # bias replicated across all 128 partitions (partition-stride 0)
            bias_sb = const_pool.tile([P, FEATURES], f32)
            b_ap = b[:]
            bias_bcast = bass.AP(
                tensor=b_ap.tensor, offset=b_ap.offset, ap=[[0, P], [1, FEATURES]]
            )
            nc.gpsimd.dma_start(out=bias_sb, in_=bias_bcast)

            ident = const_pool.tile([P, P], xdt)
            make_identity(nc, ident)

            rep_ctx = (
                tc.For_i(0, repeats, 1) if repeats > 1 else contextlib.nullcontext()
            )
            if xin_dtype == "bf16":
                # fp32->bf16 cast during DMA needs SWDGE (gpsimd); spread the
                # fp32 stores across both HWDGE rings
                in_engines, out_engines = (nc.gpsimd,), (nc.sync, nc.scalar)
            elif dma_pattern == "split":
                in_engines, out_engines = (nc.sync,), (nc.scalar,)
            elif dma_pattern == "alt2":
                in_engines, out_engines = (nc.sync, nc.scalar), (nc.scalar, nc.sync)
            elif dma_pattern == "alt3":
                in_engines = (nc.sync, nc.gpsimd, nc.scalar)
                out_engines = (nc.scalar, nc.sync, nc.gpsimd)
            elif dma_pattern == "dual":
                # both directions split as column-halves across both rings
                in_engines, out_engines = (nc.sync,), (nc.scalar,)
            elif dma_pattern in ("mono", "dual2"):
                # phase-separated HBM traffic: each ring alternates strict
                # 2MB read / 2MB write bursts; the out-DMA for tile t is
                # issued at tile t+out_delay so the ring never stalls on
                # compute (no head-of-line blocking of the next in-DMA)
                in_engines, out_engines = (nc.sync,), (nc.scalar,)
            else:
                raise ValueError(dma_pattern)

            def issue_out(rows_o, y_o):
                if dma_pattern == "mono":
                    nc.sync.dma_start(out=y[rows_o, :], in_=y_o)
                else:  # dual2
                    H = FEATURES // 2
                    nc.sync.dma_start(out=y[rows_o, :H], in_=y_o[:, :H])
                    nc.scalar.dma_start(out=y[rows_o, H:], in_=y_o[:, H:])

            with rep_ctx:
                pending = []
                for ti in range(n_tiles):
                    x_tile = x_pool.tile([P, FEATURES], xdt)
                    rows = slice(ti * P, (ti + 1) * P)
                    if ti == 0 and edge_split:
                        # split the pipeline-head DMA across BOTH rings so
                        # chunk-0 compute starts after the first quarter
                        Q = FEATURES // 4
                        for q in range(4):
                            ((nc.sync, nc.scalar)[q % 2] if edge_dual else in_engines[q % len(in_engines)]).dma_start(
                                out=x_tile[:, q * Q : (q + 1) * Q],
                                in_=x[rows, q * Q : (q + 1) * Q],
                            )
                    elif dma_pattern == "dual":
                        if xin_dtype == "bf16":
                            nc.gpsimd.dma_start(out=x_tile, in_=x[rows, :])
                        else:
                            H = FEATURES // 2
                            nc.sync.dma_start(out=x_tile[:, :H], in_=x[rows, :H])
                            nc.scalar.dma_start(out=x_tile[:, H:], in_=x[rows, H:])
                    else:
                        in_engines[ti % len(in_engines)].dma_start(
                            out=x_tile, in_=x[rows, :]
                        )

                    if dma_pattern in ("mono", "dual2") and pending and ti >= out_delay:
                        issue_out(*pending.pop(0))

                    y_tile = y_pool.tile([P, FEATURES], f32)

                    for g in range(N_CHUNKS // cg):
                        # bf16 ps_t is padded to a full 2KB PSUM bank so pool
                        # bufs never share a bank (PE-write vs ACT-read on the
                        # same bank is a fatal HW collision)
                        if xdt == f32:
                            ps_t = pst_pool.tile([P, cg * P], f32)
                        else:
                            ps_t_full = pst_pool.tile([P, 2 * cg * P], xdt)
                            ps_t = ps_t_full[:, : cg * P]
                        for k in range(cg):
                            c = g * cg + k
                            nc.tensor.transpose(
                                ps_t[:, k * P : (k + 1) * P],
                                x_tile[:, c * P : (c + 1) * P],
                                ident,
                            )
                        xt = xt_pool.tile([P, cg * P], wdt)
                        if xt_engine == "scalar":
                            nc.scalar.copy(xt, ps_t)
                        else:
                            nc.vector.tensor_copy(xt, ps_t)

                        ps_y = psy_pool.tile([P, cg * P], f32)
                        for k in range(cg):
                            c = g * cg + k
                            nc.tensor.matmul(
                                ps_y[:, k * P : (k + 1) * P],
                                xt[:, k * P : (k + 1) * P],
                                w_sb[:, c * P : (c + 1) * P],
                            )
                        # drain + fused bias add (bias varies along free dim)
                        nc.vector.tensor_add(
                            y_tile[:, g * cg * P : (g + 1) * cg * P],
                            ps_y,
                            bias_sb[:, g * cg * P : (g + 1) * cg * P],
                        )

                    # out-DMA off the input ring so both directions overlap
                    if dma_pattern in ("mono", "dual2"):
                        pending.append((rows, y_tile))
                    elif ti == n_tiles - 1 and edge_split:
                        # split the pipeline-tail DMA across BOTH rings so
                        # stores begin as soon as the first chunk groups drain
                        Q = FEATURES // 4
                        for q in range(4):
                            ((nc.scalar, nc.sync)[q % 2] if edge_dual else out_engines[q % len(out_engines)]).dma_start(
                                out=y[rows, q * Q : (q + 1) * Q],
                                in_=y_tile[:, q * Q : (q + 1) * Q],
                            )
                    elif dma_pattern == "dual":
                        H = FEATURES // 2
                        nc.scalar.dma_start(out=y[rows, :H], in_=y_tile[:, :H])
                        nc.sync.dma_start(out=y[rows, H:], in_=y_tile[:, H:])
                    else:
                        out_engines[ti % len(out_engines)].dma_start(
                            out=y[rows, :], in_=y_tile
                        )
                for rows_o, y_o in pending:
                    issue_out(rows_o, y_o)

    nc.finalize()
    return nc


def build_nc_alt2(**kw):
    return build_nc(dma_pattern="alt2", **kw)


def expand_weights(kern, mm_dtype="bf16"):
    """kernel [16, 32, 8, 8] -> [128, 32*128] chunk-major block-diagonal."""
    kern = np.asarray(kern, dtype=np.float32)
    wd = np.zeros((N_CHUNKS, P, P), dtype=np.float32)
    for c in range(N_CHUNKS):
        h = c // 2
        for j in range(16):
            bd = 16 * (c % 2) + j
            wd[c, 8 * j : 8 * j + 8, 8 * j : 8 * j + 8] = kern[h, bd]
    # [chunk, fi, fo] -> [fi, chunk*128 + fo]
    w = np.ascontiguousarray(wd.transpose(1, 0, 2).reshape(P, N_CHUNKS * P))
    if mm_dtype == "bf16":
        import ml_dtypes

        w = w.astype(ml_dtypes.bfloat16)
    return w


def reference_numpy(x, kern, bias):
    xb = np.asarray(x, np.float32).reshape(-1, NUM_HEADS, BLOCK_DIM, BLOCK_SIZE)
    k = np.asarray(kern, np.float32)
    y = np.einsum("nhbs,hbst->nhbt", xb, k) + np.asarray(bias, np.float32)
    return y.reshape(x.shape)


_LAST_EXEC_NS = None


def kernel(**inputs):
    """Full inputs in, full output out. Shards tokens across 8 cores."""
    global _LAST_EXEC_NS
    import os

    from concourse.bass_utils import run_bass_kernel_spmd

    x = np.ascontiguousarray(np.asarray(inputs["x"], dtype=np.float32))
    kern = np.asarray(inputs["kernel"], dtype=np.float32)
    bias = np.ascontiguousarray(
        np.asarray(inputs["bias"], dtype=np.float32).reshape(FEATURES)
    )

    orig_shape = x.shape
    xf = x.reshape(TOKENS_TOTAL, FEATURES)
    w = expand_weights(kern)

    if "nc" not in _NC_CACHE:
        _NC_CACHE["nc"] = build_nc()
    nc = _NC_CACHE["nc"]

    in_maps = [
        {
            "x": xf[c * TOK_PER_CORE : (c + 1) * TOK_PER_CORE],
            "w": w,
            "b": bias,
        }
        for c in range(N_CORES)
    ]

    trace = bool(os.environ.get("BASS_KERNEL_TRACE"))
    res = run_bass_kernel_spmd(nc, in_maps, list(range(N_CORES)), trace=trace)
    _LAST_EXEC_NS = res.exec_time_ns

    y = np.concatenate([r["y"] for r in res.results], axis=0)
    return y.reshape(orig_shape)



# revision 27
# speedup vs baseline: 1.0171x; 1.0087x over previous
"""Block-diagonal dense (nn_BlockDiagonalDense) Trainium2 Bass kernel.

Math: x [B=4, T=4096, F=4096] fp32; per token, features are grouped into
512 blocks of 8; each block is multiplied by its own 8x8 matrix
(kernel [16 heads, 32 blocks, 8, 8]) and bias added (bias is zeros in
setup_inputs, but we fold it in anyway).

Strategy:
  - Data-parallel over tokens across 8 cores (16384 tokens -> 2048/core).
  - Weights are expanded host-side into 32 chunks of 128x128 block-diagonal
    matrices (one per 128 consecutive features), replicated to every core
    in bf16 (tolerance is 2e-2; bf16 keeps rel err ~2e-3 and runs the PE
    at 1 cycle/row instead of fp32's 4).
  - On-chip per 128-token tile: PE transpose of each 128-feature chunk
    (fp32, via identity matmul) -> PSUM -> cast-copy to bf16 SBUF
    (ScalarE) -> PE matmul lhsT=x^T chunk (stationary bf16), rhs=W chunk
    (moving bf16) giving token-major fp32 output in PSUM -> VectorE drain
    with fused bias add -> contiguous DMA out.
  - The workload is HBM-bound (2MB in + 2MB out per tile vs ~358 GB/s
    per-core HBM); DMA pattern selection targets steady full-rate
    streaming in both directions.
"""

import sys

if "/opt/trn_rl_repo" not in sys.path:
    sys.path.insert(0, "/opt/trn_rl_repo")

import numpy as np

NUM_HEADS = 16
BLOCK_SIZE = 8
FEATURES = 4096
HEAD_DIM = FEATURES // NUM_HEADS  # 256
BLOCK_DIM = HEAD_DIM // BLOCK_SIZE  # 32

N_CORES = 8
TOKENS_TOTAL = 4 * 4096  # 16384
TOK_PER_CORE = TOKENS_TOTAL // N_CORES  # 2048

P = 128  # partitions
N_CHUNKS = FEATURES // P  # 32 chunks of 128 features
CG = 4  # chunks per group (512 output cols per PSUM bank)

_NC_CACHE = {}


def build_nc(
    tok_per_core=TOK_PER_CORE,
    repeats=1,
    dma_pattern="split",
    edge_split=True,
    xt_engine="scalar",
    edge_dual=False,
    cg=CG,
    pst_bufs=3,
    psy_bufs=3,
    xbufs=4,
    ybufs=4,
    xtbufs=4,
    mm_dtype="bf16",
    xin_dtype="f32",
    out_delay=2,
    out_batch=3,
    fold=1,
):
    """Build the Bass program for one core processing [tok_per_core, 4096].

    repeats>1 wraps the whole body in a hardware loop doing identical work
    (same inputs, same outputs) -- used only for slope-based device timing.

    dma_pattern: "split" = x on SP ring / y on ACT ring;
                 "alt2"  = both rings alternate directions per tile;
                 "alt3"  = SP + ACT + SWDGE(gpsimd) rotate.
    """
    import contextlib

    import concourse.bass as bass
    import concourse.mybir as mybir
    from concourse import bacc
    from concourse.masks import make_identity
    from concourse.tile import TileContext

    f32 = mybir.dt.float32
    wdt = mybir.dt.bfloat16 if mm_dtype == "bf16" else f32
    xdt = mybir.dt.bfloat16 if xin_dtype == "bf16" else f32
    nc = bacc.Bacc(None, target_bir_lowering=False)

    # fold>1 reinterprets the row-major [tok, 4096] block as
    # [tok/fold, 4096*fold] (fold tokens per row) -> bigger DMAs, same math;
    # chunk c uses weight chunk (c % 32) and bias column block (c % 32).
    ROWS = tok_per_core // fold
    COLS = FEATURES * fold
    NCH = N_CHUNKS * fold
    x = nc.declare_dram_parameter("x", [ROWS, COLS], f32, isOutput=False)
    # w: [128 (fi within chunk), 32*128 (chunk-major, fo within chunk)]
    w = nc.declare_dram_parameter("w", [P, N_CHUNKS * P], wdt, isOutput=False)
    b = nc.declare_dram_parameter("b", [FEATURES], f32, isOutput=False)
    y = nc.declare_dram_parameter("y", [ROWS, COLS], f32, isOutput=True)

    n_tiles = ROWS // P

    with TileContext(nc) as tc:
        with (
            tc.tile_pool(name="const", bufs=1) as const_pool,
            tc.tile_pool(name="xin", bufs=xbufs) as x_pool,
            tc.tile_pool(name="yout", bufs=ybufs) as y_pool,
            tc.tile_pool(name="xt", bufs=xtbufs) as xt_pool,
            tc.tile_pool(name="pst", bufs=pst_bufs, space="PSUM") as pst_pool,
            tc.tile_pool(name="psy", bufs=psy_bufs, space="PSUM") as psy_pool,
        ):
            # w on the ACT ring: keeps tile-0's x DMA unqueued on the SP ring
            w_sb = const_pool.tile([P, N_CHUNKS * P], wdt)
            nc.scalar.dma_start(out=w_sb, in_=w[:, :])

            # bias replicated across all 128 partitions (partition-stride 0)
            bias_sb = const_pool.tile([P, FEATURES], f32)
            b_ap = b[:]
            bias_bcast = bass.AP(
                tensor=b_ap.tensor, offset=b_ap.offset, ap=[[0, P], [1, FEATURES]]
            )
            nc.gpsimd.dma_start(out=bias_sb, in_=bias_bcast)

            ident = const_pool.tile([P, P], xdt)
            make_identity(nc, ident)

            rep_ctx = (
                tc.For_i(0, repeats, 1) if repeats > 1 else contextlib.nullcontext()
            )
            if xin_dtype == "bf16":
                # fp32->bf16 cast during DMA needs SWDGE (gpsimd); spread the
                # fp32 stores across both HWDGE rings
                in_engines, out_engines = (nc.gpsimd,), (nc.sync, nc.scalar)
            elif dma_pattern == "split":
                in_engines, out_engines = (nc.sync,), (nc.scalar,)
            elif dma_pattern == "alt2":
                in_engines, out_engines = (nc.sync, nc.scalar), (nc.scalar, nc.sync)
            elif dma_pattern == "alt3":
                in_engines = (nc.sync, nc.gpsimd, nc.scalar)
                out_engines = (nc.scalar, nc.sync, nc.gpsimd)
            elif dma_pattern == "dual":
                # both directions split as column-halves across both rings
                in_engines, out_engines = (nc.sync,), (nc.scalar,)
            elif dma_pattern in ("mono", "dual2", "split2", "mono_b"):
                # delayed-issue patterns: the out-DMA for tile t is issued
                # at tile t+out_delay so no engine ever stalls waiting for
                # tile t's last bias-add before its next trigger ("mono" =
                # both directions on SP ring, "dual2" = column-halves on
                # both rings, "split2" = classic split ring assignment)
                in_engines, out_engines = (nc.sync,), (nc.scalar,)
            else:
                raise ValueError(dma_pattern)

            def issue_out(rows_o, y_o):
                if dma_pattern in ("mono", "mono_b"):
                    nc.sync.dma_start(out=y[rows_o, :], in_=y_o)
                elif dma_pattern == "split2":
                    nc.scalar.dma_start(out=y[rows_o, :], in_=y_o)
                else:  # dual2
                    H = COLS // 2
                    nc.sync.dma_start(out=y[rows_o, :H], in_=y_o[:, :H])
                    nc.scalar.dma_start(out=y[rows_o, H:], in_=y_o[:, H:])

            with rep_ctx:
                pending = []
                for ti in range(n_tiles):
                    x_tile = x_pool.tile([P, COLS], xdt)
                    rows = slice(ti * P, (ti + 1) * P)
                    if ti == 0 and edge_split:
                        # split the pipeline-head DMA across BOTH rings so
                        # chunk-0 compute starts after the first quarter
                        Q = COLS // 4
                        for q in range(4):
                            ((nc.sync, nc.scalar)[q % 2] if edge_dual else in_engines[q % len(in_engines)]).dma_start(
                                out=x_tile[:, q * Q : (q + 1) * Q],
                                in_=x[rows, q * Q : (q + 1) * Q],
                            )
                    elif dma_pattern == "dual":
                        if xin_dtype == "bf16":
                            nc.gpsimd.dma_start(out=x_tile, in_=x[rows, :])
                        else:
                            H = COLS // 2
                            nc.sync.dma_start(out=x_tile[:, :H], in_=x[rows, :H])
                            nc.scalar.dma_start(out=x_tile[:, H:], in_=x[rows, H:])
                    else:
                        in_engines[ti % len(in_engines)].dma_start(
                            out=x_tile, in_=x[rows, :]
                        )

                    if dma_pattern == "mono_b":
                        # batched: 3-tile read bursts alternate with 3-tile
                        # write bursts on one ring (longer same-direction
                        # HBM runs; probed ~2% faster than fine interleave)
                        if ti % out_batch == out_batch - 1 and len(pending) >= out_batch:
                            for _ in range(out_batch):
                                issue_out(*pending.pop(0))
                    elif dma_pattern in ("mono", "dual2", "split2") and pending and ti >= out_delay:
                        issue_out(*pending.pop(0))

                    y_tile = y_pool.tile([P, COLS], f32)

                    for g in range(NCH // cg):
                        # bf16 ps_t is padded to a full 2KB PSUM bank so pool
                        # bufs never share a bank (PE-write vs ACT-read on the
                        # same bank is a fatal HW collision)
                        if xdt == f32:
                            ps_t = pst_pool.tile([P, cg * P], f32)
                        else:
                            ps_t_full = pst_pool.tile([P, 2 * cg * P], xdt)
                            ps_t = ps_t_full[:, : cg * P]
                        for k in range(cg):
                            c = g * cg + k
                            nc.tensor.transpose(
                                ps_t[:, k * P : (k + 1) * P],
                                x_tile[:, c * P : (c + 1) * P],
                                ident,
                            )
                        xt = xt_pool.tile([P, cg * P], wdt)
                        if xt_engine == "scalar":
                            nc.scalar.copy(xt, ps_t)
                        else:
                            nc.vector.tensor_copy(xt, ps_t)

                        ps_y = psy_pool.tile([P, cg * P], f32)
                        for k in range(cg):
                            c = g * cg + k
                            nc.tensor.matmul(
                                ps_y[:, k * P : (k + 1) * P],
                                xt[:, k * P : (k + 1) * P],
                                w_sb[:, (c % N_CHUNKS) * P : (c % N_CHUNKS + 1) * P],
                            )
                        # drain + fused bias add (bias varies along free dim)
                        boff = (g * cg * P) % FEATURES
                        nc.vector.tensor_add(
                            y_tile[:, g * cg * P : (g + 1) * cg * P],
                            ps_y,
                            bias_sb[:, boff : boff + cg * P],
                        )

                    # out-DMA off the input ring so both directions overlap
                    if dma_pattern in ("mono", "dual2", "split2", "mono_b"):
                        pending.append((rows, y_tile))
                    elif ti == n_tiles - 1 and edge_split:
                        # split the pipeline-tail DMA across BOTH rings so
                        # stores begin as soon as the first chunk groups drain
                        Q = COLS // 4
                        for q in range(4):
                            ((nc.scalar, nc.sync)[q % 2] if edge_dual else out_engines[q % len(out_engines)]).dma_start(
                                out=y[rows, q * Q : (q + 1) * Q],
                                in_=y_tile[:, q * Q : (q + 1) * Q],
                            )
                    elif dma_pattern == "dual":
                        H = COLS // 2
                        nc.scalar.dma_start(out=y[rows, :H], in_=y_tile[:, :H])
                        nc.sync.dma_start(out=y[rows, H:], in_=y_tile[:, H:])
                    else:
                        out_engines[ti % len(out_engines)].dma_start(
                            out=y[rows, :], in_=y_tile
                        )
                for rows_o, y_o in pending:
                    issue_out(rows_o, y_o)

    nc.finalize()
    return nc


def build_nc_alt2(**kw):
    return build_nc(dma_pattern="alt2", **kw)


def expand_weights(kern, mm_dtype="bf16"):
    """kernel [16, 32, 8, 8] -> [128, 32*128] chunk-major block-diagonal."""
    kern = np.asarray(kern, dtype=np.float32)
    wd = np.zeros((N_CHUNKS, P, P), dtype=np.float32)
    for c in range(N_CHUNKS):
        h = c // 2
        for j in range(16):
            bd = 16 * (c % 2) + j
            wd[c, 8 * j : 8 * j + 8, 8 * j : 8 * j + 8] = kern[h, bd]
    # [chunk, fi, fo] -> [fi, chunk*128 + fo]
    w = np.ascontiguousarray(wd.transpose(1, 0, 2).reshape(P, N_CHUNKS * P))
    if mm_dtype == "bf16":
        import ml_dtypes

        w = w.astype(ml_dtypes.bfloat16)
    return w


def reference_numpy(x, kern, bias):
    xb = np.asarray(x, np.float32).reshape(-1, NUM_HEADS, BLOCK_DIM, BLOCK_SIZE)
    k = np.asarray(kern, np.float32)
    y = np.einsum("nhbs,hbst->nhbt", xb, k) + np.asarray(bias, np.float32)
    return y.reshape(x.shape)


_LAST_EXEC_NS = None


def kernel(**inputs):
    """Full inputs in, full output out. Shards tokens across 8 cores."""
    global _LAST_EXEC_NS
    import os

    from concourse.bass_utils import run_bass_kernel_spmd

    x = np.ascontiguousarray(np.asarray(inputs["x"], dtype=np.float32))
    kern = np.asarray(inputs["kernel"], dtype=np.float32)
    bias = np.ascontiguousarray(
        np.asarray(inputs["bias"], dtype=np.float32).reshape(FEATURES)
    )

    orig_shape = x.shape
    xf = x.reshape(TOKENS_TOTAL, FEATURES)
    w = expand_weights(kern)

    if "nc" not in _NC_CACHE:
        _NC_CACHE["nc"] = build_nc()
    nc = _NC_CACHE["nc"]

    in_maps = [
        {
            "x": xf[c * TOK_PER_CORE : (c + 1) * TOK_PER_CORE],
            "w": w,
            "b": bias,
        }
        for c in range(N_CORES)
    ]

    trace = bool(os.environ.get("BASS_KERNEL_TRACE"))
    res = run_bass_kernel_spmd(nc, in_maps, list(range(N_CORES)), trace=trace)
    _LAST_EXEC_NS = res.exec_time_ns

    y = np.concatenate([r["y"] for r in res.results], axis=0)
    return y.reshape(orig_shape)



# revision 29
# speedup vs baseline: 1.0245x; 1.0073x over previous
"""Block-diagonal dense (nn_BlockDiagonalDense) Trainium2 Bass kernel.

Math: x [B=4, T=4096, F=4096] fp32; per token, features are grouped into
512 blocks of 8; each block is multiplied by its own 8x8 matrix
(kernel [16 heads, 32 blocks, 8, 8]) and bias added (bias is zeros in
setup_inputs, but we fold it in anyway).

Strategy:
  - Data-parallel over tokens across 8 cores (16384 tokens -> 2048/core).
  - Weights are expanded host-side into 32 chunks of 128x128 block-diagonal
    matrices (one per 128 consecutive features), replicated to every core
    in bf16 (tolerance is 2e-2; bf16 keeps rel err ~2e-3 and runs the PE
    at 1 cycle/row instead of fp32's 4).
  - On-chip per 128-token tile: PE transpose of each 128-feature chunk
    (fp32, via identity matmul) -> PSUM -> cast-copy to bf16 SBUF
    (ScalarE) -> PE matmul lhsT=x^T chunk (stationary bf16), rhs=W chunk
    (moving bf16) giving token-major fp32 output in PSUM -> VectorE drain
    with fused bias add -> contiguous DMA out.
  - The workload is HBM-bound (2MB in + 2MB out per tile vs ~358 GB/s
    per-core HBM); DMA pattern selection targets steady full-rate
    streaming in both directions.
"""

import sys

if "/opt/trn_rl_repo" not in sys.path:
    sys.path.insert(0, "/opt/trn_rl_repo")

import numpy as np

NUM_HEADS = 16
BLOCK_SIZE = 8
FEATURES = 4096
HEAD_DIM = FEATURES // NUM_HEADS  # 256
BLOCK_DIM = HEAD_DIM // BLOCK_SIZE  # 32

N_CORES = 8
TOKENS_TOTAL = 4 * 4096  # 16384
TOK_PER_CORE = TOKENS_TOTAL // N_CORES  # 2048

P = 128  # partitions
N_CHUNKS = FEATURES // P  # 32 chunks of 128 features
CG = 4  # chunks per group (512 output cols per PSUM bank)

_NC_CACHE = {}


def build_nc(
    tok_per_core=TOK_PER_CORE,
    repeats=1,
    dma_pattern="split",
    edge_split=False,
    xt_engine="scalar",
    edge_dual=False,
    cg=CG,
    pst_bufs=3,
    psy_bufs=3,
    xbufs=4,
    ybufs=4,
    xtbufs=4,
    mm_dtype="bf16",
    xin_dtype="f32",
    out_delay=2,
    out_batch=3,
    fold=1,
):
    """Build the Bass program for one core processing [tok_per_core, 4096].

    repeats>1 wraps the whole body in a hardware loop doing identical work
    (same inputs, same outputs) -- used only for slope-based device timing.

    dma_pattern: "split" = x on SP ring / y on ACT ring;
                 "alt2"  = both rings alternate directions per tile;
                 "alt3"  = SP + ACT + SWDGE(gpsimd) rotate.
    """
    import contextlib

    import concourse.bass as bass
    import concourse.mybir as mybir
    from concourse import bacc
    from concourse.masks import make_identity
    from concourse.tile import TileContext

    f32 = mybir.dt.float32
    wdt = mybir.dt.bfloat16 if mm_dtype == "bf16" else f32
    xdt = mybir.dt.bfloat16 if xin_dtype == "bf16" else f32
    nc = bacc.Bacc(None, target_bir_lowering=False)

    # fold>1 reinterprets the row-major [tok, 4096] block as
    # [tok/fold, 4096*fold] (fold tokens per row) -> bigger DMAs, same math;
    # chunk c uses weight chunk (c % 32) and bias column block (c % 32).
    ROWS = tok_per_core // fold
    COLS = FEATURES * fold
    NCH = N_CHUNKS * fold
    x = nc.declare_dram_parameter("x", [ROWS, COLS], f32, isOutput=False)
    # w: [128 (fi within chunk), 32*128 (chunk-major, fo within chunk)]
    w = nc.declare_dram_parameter("w", [P, N_CHUNKS * P], wdt, isOutput=False)
    b = nc.declare_dram_parameter("b", [FEATURES], f32, isOutput=False)
    y = nc.declare_dram_parameter("y", [ROWS, COLS], f32, isOutput=True)

    n_tiles = ROWS // P

    with TileContext(nc) as tc:
        with (
            tc.tile_pool(name="const", bufs=1) as const_pool,
            tc.tile_pool(name="xin", bufs=xbufs) as x_pool,
            tc.tile_pool(name="yout", bufs=ybufs) as y_pool,
            tc.tile_pool(name="xt", bufs=xtbufs) as xt_pool,
            tc.tile_pool(name="pst", bufs=pst_bufs, space="PSUM") as pst_pool,
            tc.tile_pool(name="psy", bufs=psy_bufs, space="PSUM") as psy_pool,
        ):
            # w on the ACT ring: keeps tile-0's x DMA unqueued on the SP ring
            w_sb = const_pool.tile([P, N_CHUNKS * P], wdt)
            nc.scalar.dma_start(out=w_sb, in_=w[:, :])

            # bias replicated across all 128 partitions (partition-stride 0)
            bias_sb = const_pool.tile([P, FEATURES], f32)
            b_ap = b[:]
            bias_bcast = bass.AP(
                tensor=b_ap.tensor, offset=b_ap.offset, ap=[[0, P], [1, FEATURES]]
            )
            nc.gpsimd.dma_start(out=bias_sb, in_=bias_bcast)

            ident = const_pool.tile([P, P], xdt)
            make_identity(nc, ident)

            rep_ctx = (
                tc.For_i(0, repeats, 1) if repeats > 1 else contextlib.nullcontext()
            )
            if xin_dtype == "bf16":
                # fp32->bf16 cast during DMA needs SWDGE (gpsimd); spread the
                # fp32 stores across both HWDGE rings
                in_engines, out_engines = (nc.gpsimd,), (nc.sync, nc.scalar)
            elif dma_pattern == "split":
                in_engines, out_engines = (nc.sync,), (nc.scalar,)
            elif dma_pattern == "alt2":
                in_engines, out_engines = (nc.sync, nc.scalar), (nc.scalar, nc.sync)
            elif dma_pattern == "alt3":
                in_engines = (nc.sync, nc.gpsimd, nc.scalar)
                out_engines = (nc.scalar, nc.sync, nc.gpsimd)
            elif dma_pattern == "dual":
                # both directions split as column-halves across both rings
                in_engines, out_engines = (nc.sync,), (nc.scalar,)
            elif dma_pattern in ("mono", "dual2", "split2", "mono_b", "sw_b"):
                # delayed-issue patterns: the out-DMA for tile t is issued
                # at tile t+out_delay so no engine ever stalls waiting for
                # tile t's last bias-add before its next trigger ("mono" =
                # both directions on SP ring, "dual2" = column-halves on
                # both rings, "split2" = classic split ring assignment)
                in_engines, out_engines = (nc.sync,), (nc.scalar,)
            else:
                raise ValueError(dma_pattern)

            def issue_out(rows_o, y_o):
                if dma_pattern == "sw_b":
                    # single SWDGE queue for both directions: strict
                    # same-direction bursts at the HBM (probed ~2% faster)
                    nc.gpsimd.dma_start(out=y[rows_o, :], in_=y_o)
                elif dma_pattern in ("mono", "mono_b"):
                    nc.sync.dma_start(out=y[rows_o, :], in_=y_o)
                elif dma_pattern == "split2":
                    nc.scalar.dma_start(out=y[rows_o, :], in_=y_o)
                else:  # dual2
                    H = COLS // 2
                    nc.sync.dma_start(out=y[rows_o, :H], in_=y_o[:, :H])
                    nc.scalar.dma_start(out=y[rows_o, H:], in_=y_o[:, H:])

            with rep_ctx:
                pending = []
                for ti in range(n_tiles):
                    x_tile = x_pool.tile([P, COLS], xdt)
                    rows = slice(ti * P, (ti + 1) * P)
                    if ti == 0 and edge_split:
                        # split the pipeline-head DMA across BOTH rings so
                        # chunk-0 compute starts after the first quarter
                        Q = COLS // 4
                        for q in range(4):
                            ((nc.sync, nc.scalar)[q % 2] if edge_dual else in_engines[q % len(in_engines)]).dma_start(
                                out=x_tile[:, q * Q : (q + 1) * Q],
                                in_=x[rows, q * Q : (q + 1) * Q],
                            )
                    elif dma_pattern == "dual":
                        if xin_dtype == "bf16":
                            nc.gpsimd.dma_start(out=x_tile, in_=x[rows, :])
                        else:
                            H = COLS // 2
                            nc.sync.dma_start(out=x_tile[:, :H], in_=x[rows, :H])
                            nc.scalar.dma_start(out=x_tile[:, H:], in_=x[rows, H:])
                    else:
                        in_engines[ti % len(in_engines)].dma_start(
                            out=x_tile, in_=x[rows, :]
                        )

                    if dma_pattern in ("mono_b", "sw_b"):
                        # batched: 3-tile read bursts alternate with 3-tile
                        # write bursts on one ring (longer same-direction
                        # HBM runs; probed ~2% faster than fine interleave)
                        if ti % out_batch == out_batch - 1 and len(pending) >= out_batch:
                            for _ in range(out_batch):
                                issue_out(*pending.pop(0))
                    elif dma_pattern in ("mono", "dual2", "split2") and pending and ti >= out_delay:
                        issue_out(*pending.pop(0))

                    y_tile = y_pool.tile([P, COLS], f32)

                    for g in range(NCH // cg):
                        # bf16 ps_t is padded to a full 2KB PSUM bank so pool
                        # bufs never share a bank (PE-write vs ACT-read on the
                        # same bank is a fatal HW collision)
                        if xdt == f32:
                            ps_t = pst_pool.tile([P, cg * P], f32)
                        else:
                            ps_t_full = pst_pool.tile([P, 2 * cg * P], xdt)
                            ps_t = ps_t_full[:, : cg * P]
                        for k in range(cg):
                            c = g * cg + k
                            nc.tensor.transpose(
                                ps_t[:, k * P : (k + 1) * P],
                                x_tile[:, c * P : (c + 1) * P],
                                ident,
                            )
                        xt = xt_pool.tile([P, cg * P], wdt)
                        if xt_engine == "scalar":
                            nc.scalar.copy(xt, ps_t)
                        else:
                            nc.vector.tensor_copy(xt, ps_t)

                        ps_y = psy_pool.tile([P, cg * P], f32)
                        for k in range(cg):
                            c = g * cg + k
                            nc.tensor.matmul(
                                ps_y[:, k * P : (k + 1) * P],
                                xt[:, k * P : (k + 1) * P],
                                w_sb[:, (c % N_CHUNKS) * P : (c % N_CHUNKS + 1) * P],
                            )
                        # drain + fused bias add (bias varies along free dim)
                        boff = (g * cg * P) % FEATURES
                        nc.vector.tensor_add(
                            y_tile[:, g * cg * P : (g + 1) * cg * P],
                            ps_y,
                            bias_sb[:, boff : boff + cg * P],
                        )

                    # out-DMA off the input ring so both directions overlap
                    if dma_pattern in ("mono", "dual2", "split2", "mono_b", "sw_b"):
                        pending.append((rows, y_tile))
                    elif ti == n_tiles - 1 and edge_split:
                        # split the pipeline-tail DMA across BOTH rings so
                        # stores begin as soon as the first chunk groups drain
                        Q = COLS // 4
                        for q in range(4):
                            ((nc.scalar, nc.sync)[q % 2] if edge_dual else out_engines[q % len(out_engines)]).dma_start(
                                out=y[rows, q * Q : (q + 1) * Q],
                                in_=y_tile[:, q * Q : (q + 1) * Q],
                            )
                    elif dma_pattern == "dual":
                        H = COLS // 2
                        nc.scalar.dma_start(out=y[rows, :H], in_=y_tile[:, :H])
                        nc.sync.dma_start(out=y[rows, H:], in_=y_tile[:, H:])
                    else:
                        out_engines[ti % len(out_engines)].dma_start(
                            out=y[rows, :], in_=y_tile
                        )
                for rows_o, y_o in pending:
                    issue_out(rows_o, y_o)

    nc.finalize()
    return nc


def build_nc_alt2(**kw):
    return build_nc(dma_pattern="alt2", **kw)


def expand_weights(kern, mm_dtype="bf16"):
    """kernel [16, 32, 8, 8] -> [128, 32*128] chunk-major block-diagonal."""
    kern = np.asarray(kern, dtype=np.float32)
    wd = np.zeros((N_CHUNKS, P, P), dtype=np.float32)
    for c in range(N_CHUNKS):
        h = c // 2
        for j in range(16):
            bd = 16 * (c % 2) + j
            wd[c, 8 * j : 8 * j + 8, 8 * j : 8 * j + 8] = kern[h, bd]
    # [chunk, fi, fo] -> [fi, chunk*128 + fo]
    w = np.ascontiguousarray(wd.transpose(1, 0, 2).reshape(P, N_CHUNKS * P))
    if mm_dtype == "bf16":
        import ml_dtypes

        w = w.astype(ml_dtypes.bfloat16)
    return w


def reference_numpy(x, kern, bias):
    xb = np.asarray(x, np.float32).reshape(-1, NUM_HEADS, BLOCK_DIM, BLOCK_SIZE)
    k = np.asarray(kern, np.float32)
    y = np.einsum("nhbs,hbst->nhbt", xb, k) + np.asarray(bias, np.float32)
    return y.reshape(x.shape)


_LAST_EXEC_NS = None


def kernel(**inputs):
    """Full inputs in, full output out. Shards tokens across 8 cores."""
    global _LAST_EXEC_NS
    import os

    from concourse.bass_utils import run_bass_kernel_spmd

    x = np.ascontiguousarray(np.asarray(inputs["x"], dtype=np.float32))
    kern = np.asarray(inputs["kernel"], dtype=np.float32)
    bias = np.ascontiguousarray(
        np.asarray(inputs["bias"], dtype=np.float32).reshape(FEATURES)
    )

    orig_shape = x.shape
    xf = x.reshape(TOKENS_TOTAL, FEATURES)
    w = expand_weights(kern)

    if "nc" not in _NC_CACHE:
        _NC_CACHE["nc"] = build_nc()
    nc = _NC_CACHE["nc"]

    in_maps = [
        {
            "x": xf[c * TOK_PER_CORE : (c + 1) * TOK_PER_CORE],
            "w": w,
            "b": bias,
        }
        for c in range(N_CORES)
    ]

    trace = bool(os.environ.get("BASS_KERNEL_TRACE"))
    res = run_bass_kernel_spmd(nc, in_maps, list(range(N_CORES)), trace=trace)
    _LAST_EXEC_NS = res.exec_time_ns

    y = np.concatenate([r["y"] for r in res.results], axis=0)
    return y.reshape(orig_shape)

